# revision 25
# baseline (speedup 1.0000x reference)
"""Causal multi-head self-attention (B=2, S=2048, D=1024, H=16, Dh=64, RoPE)
as a Bass/Tile kernel on 8 Trainium2 NeuronCores.

Sharding: 2-way batch x 4-way head-group tensor parallel.
  core c: batch b = c // 4, head group g = c % 4 (heads 4g..4g+3).
  Wq/Wk/Wv split column-wise by head (rows of W since y = x @ W.T),
  Wo split row-wise; the 4 partial outputs per batch are summed on host.

Per-core layout choices:
  - x is passed transposed (xT: d_model on partitions) so Q^T/K^T come out of
    the projection matmuls directly in (head_dim, seq) layout, which is what
    the scores matmul (contraction over head_dim on partitions) needs.
  - Wq/Wk rows are de-interleaved per head (even pair-elements then odd) so
    RoPE becomes ops on contiguous 32-partition halves. Scores are invariant
    to a consistent permutation of Q/K features, so nothing is un-permuted.
  - scores are computed transposed (keys on partitions, queries free) so the
    P @ V matmul consumes exp(scores) directly with no transposes. Softmax
    skips the max-subtraction (scores are bounded ~|10| here) and the
    denominator comes from a ones-column appended to V.
  - causal mask: fully-masked key blocks are skipped; diagonal blocks get
    -1e9 added via an extra identity @ shifted-step-matrix matmul into the
    same PSUM accumulation, before exp.
  - all matmuls run in bf16 (measured 216 ns vs fp32r's 290 ns per N=512
    matmul on HW); accumulation stays fp32 in PSUM. Host converts x/W to
    bf16; on-device producers (RoPE adds, exp, V/atten-out copies) write
    bf16 directly so no extra conversion instructions exist.
  - softmax denominators use the single-op reciprocal_approx_fast (~18
    correct bits) instead of the iterative DVE reciprocal, which profiled
    at ~4 us per (1,512) row and kept the PE idle long enough to re-throttle
    the HAM clock gate.
  - a short burst of zero matmuls at kernel start warms the PE clock gate
    (cold K=4/8 at 1.2 GHz -> warm 8/8 at 2.4 GHz) while the weight DMAs
    stream in.
"""
import sys

sys.path.insert(0, "/opt/trn_rl_repo")

import numpy as np
import ml_dtypes

import concourse.bass as bass
import concourse.tile as tile
from concourse import bacc, mybir
from concourse.bass_utils import run_bass_kernel_spmd

F32 = mybir.dt.float32
BF16 = mybir.dt.bfloat16
NP_BF16 = ml_dtypes.bfloat16

B = 2
S = 2048
D = 1024
H = 16
DH = 64
NCORES = 8
NGROUPS = 4           # head groups (tensor parallel)
HL = H // NGROUPS     # heads per core = 4
F = HL * DH           # local features per core = 256
SC = 512              # seq chunk (free dim of most matmuls)
NSC = S // SC         # 4
KB = 128              # key block (partition dim of scoresT)
THETA = 10000.0
NEG = -1.0e9
SWAP16 = list(range(16, 32)) + list(range(16))
NWARM = 36            # HAM warm-up matmuls at kernel start: long enough to
                      # bridge the PE from the preamble through the staggered
                      # weight/x DMA arrivals (~24 us) so the clock gate never
                      # re-throttles before steady state; the dep-free fillers
                      # interleave between the DMA-gated projection k-groups
NTAIL = 10            # dummy matmuls emitted before the last Wo block: they
                      # fill the PE idle window while the last chunk's
                      # softmax-normalization chains drain, so the clock gate
                      # stays at 2.4 GHz for the final Wo matmuls


def build_nc(repeat=1, debug=False):
    nc = bacc.Bacc("TRN2", target_bir_lowering=False)

    xT = nc.dram_tensor("xT", (D, S), BF16, kind="ExternalInput")
    wqT = nc.dram_tensor("wqT", (D, F), BF16, kind="ExternalInput")
    wkT = nc.dram_tensor("wkT", (D, F), BF16, kind="ExternalInput")
    wvT = nc.dram_tensor("wvT", (D, F), BF16, kind="ExternalInput")
    woT = nc.dram_tensor("woT", (F, D), BF16, kind="ExternalInput")
    cc = nc.dram_tensor("cc", (64, S), BF16, kind="ExternalInput")
    ss = nc.dram_tensor("ss", (64, S), BF16, kind="ExternalInput")
    gmask = nc.dram_tensor("gmask", (128, 2 * SC), BF16, kind="ExternalInput")
    eye = nc.dram_tensor("eye", (128, 128), BF16, kind="ExternalInput")
    # bf16 output halves the 8 MB/rep of DRAM write traffic that otherwise
    # bounds the chunk-3/tail drain; the host upcasts and sums in f64
    out = nc.dram_tensor("out", (S, D), BF16, kind="ExternalOutput")
    if debug:
        qtrd = nc.dram_tensor("qtrd", (2, 128, S), BF16, kind="ExternalOutput")
        ktrd = nc.dram_tensor("ktrd", (2, 128, S), BF16, kind="ExternalOutput")
        aotd = nc.dram_tensor("aotd", (2, 128, S), BF16, kind="ExternalOutput")
        vexd = nc.dram_tensor("vexd", (16, 128, HL * 65), BF16, kind="ExternalOutput")

    KC = D // 128  # 8 contraction chunks

    with tile.TileContext(nc) as tc:
        with (
            tc.tile_pool(name="consts", bufs=1) as consts,
            tc.tile_pool(name="persist", bufs=1) as persist,
            tc.tile_pool(name="xs", bufs=2) as xs_pool,
            tc.tile_pool(name="rope", bufs=3) as rope_pool,
            tc.tile_pool(name="pp", bufs=5) as p_pool,
            tc.tile_pool(name="stg", bufs=3) as stg_pool,
            tc.tile_pool(name="og", bufs=4) as out_pool,
            tc.tile_pool(name="mm_ps", bufs=2, space="PSUM") as mm_ps,
            tc.tile_pool(name="sc_ps", bufs=2, space="PSUM") as sc_ps,
            tc.tile_pool(name="pv_ps", bufs=2, space="PSUM") as pv_ps,
        ):
            # ---- HAM warm-up: keep the PE busy while the weight DMAs land ----
            zt = consts.tile([128, SC], BF16)
            nc.vector.memset(zt, 0.0)
            for w in range(NWARM):
                wps = mm_ps.tile([128, SC], F32, name="wps", tag="mm")
                nc.tensor.matmul(wps, zt[:, 0:128], zt, start=True, stop=True)

            # ---- constants / weights ----
            # spread the startup loads across the per-engine DMA queues (each
            # dma_start has ~1 us first-byte latency, and a single queue
            # serializes them: profiled 40 us of everything-waiting), ordered
            # by first use within each queue.
            wq_sb = consts.tile([128, KC, F], BF16)
            wk_sb = consts.tile([128, KC, F], BF16)
            wv_sb = consts.tile([128, KC, F], BF16)
            wo_sb = consts.tile([128, 2, D], BF16)
            cc_sb = consts.tile([128, S], BF16)
            ss_sb = consts.tile([128, S], BF16)
            gm_sb = consts.tile([128, 2 * SC], BF16)
            eye_sb = consts.tile([128, 128], BF16)
            xt0 = xs_pool.tile([128, KC, SC], BF16, name="xt0", tag="xt")
            # startup-critical loads only; cc/ss second halves and wo are
            # issued inside the loop (after attn(0)) so their descriptors
            # don't steal DMA bandwidth from wq/xt0/wk. Spread across FOUR
            # engine queues (sync/gpsimd/scalar/vector) so the weight loads
            # run in parallel: sync carries only wq, vector carries wk/wv +
            # eye/gm, scalar the rope tables, gpsimd the x chunks.
            # cc/ss rows 0-63 == 64-127, so DRAM is read once per half and
            # duplicated into both partition halves.
            xTr = xT.rearrange("(kc p) s -> p kc s", p=128)
            wqr = wqT.rearrange("(kc p) f -> p kc f", p=128)
            wkr = wkT.rearrange("(kc p) f -> p kc f", p=128)
            # first k-halves of wq/xt0/wk land first so chunk-0 projection
            # can start on k 0..3 while k 4..7 streams in; the h1 loads are
            # split in two dma_starts each so their descriptors spread over
            # more DMA engines and the first bytes land sooner
            nc.sync.dma_start(out=wq_sb[:, 0:KC // 4, :], in_=wqr[:, 0:KC // 4, :])
            nc.gpsimd.dma_start(out=xt0[:, 0:KC // 4, :], in_=xTr[:, 0:KC // 4, 0:SC])
            nc.sync.dma_start(out=wq_sb[:, KC // 4:KC // 2, :], in_=wqr[:, KC // 4:KC // 2, :])
            nc.gpsimd.dma_start(out=xt0[:, KC // 4:KC // 2, :], in_=xTr[:, KC // 4:KC // 2, 0:SC])
            nc.sync.dma_start(out=wq_sb[:, KC // 2:, :], in_=wqr[:, KC // 2:, :])
            nc.gpsimd.dma_start(out=xt0[:, KC // 2:, :], in_=xTr[:, KC // 2:, 0:SC])
            nc.sync.dma_start(out=wk_sb[:, 0:KC // 2, :], in_=wkr[:, 0:KC // 2, :])
            nc.sync.dma_start(out=wk_sb[:, KC // 2:, :], in_=wkr[:, KC // 2:, :])
            # wo is consumed from window 1 on (Wo(0) is spread forward into
            # the PE-idle early windows), so it loads at startup, not late
            nc.sync.dma_start(out=wo_sb, in_=woT.rearrange("(ft p) d -> p ft d", p=128))
            nc.scalar.dma_start(out=cc_sb[0:64, 0:S // 2], in_=cc[:, 0:S // 2])
            nc.scalar.dma_start(out=ss_sb[0:64, 0:S // 2], in_=ss[:, 0:S // 2])
            nc.scalar.dma_start(out=cc_sb[64:128, 0:S // 2], in_=cc[:, 0:S // 2])
            nc.scalar.dma_start(out=ss_sb[64:128, 0:S // 2], in_=ss[:, 0:S // 2])
            nc.scalar.dma_start(out=eye_sb, in_=eye[:, :])
            nc.scalar.dma_start(out=gm_sb, in_=gmask[:, :])
            nc.scalar.dma_start(out=wv_sb, in_=wvT.rearrange("(kc p) f -> p kc f", p=128))

            def emit_late_consts():
                for half in (slice(0, 64), slice(64, 128)):
                    nc.scalar.dma_start(out=cc_sb[half, S // 2:], in_=cc[:, S // 2:])
                    nc.scalar.dma_start(out=ss_sb[half, S // 2:], in_=ss[:, S // 2:])

            # ---- persistent activations ----
            qtr = [persist.tile([128, S], BF16, name=f"qtr{i}", tag=f"qtr{i}") for i in range(2)]
            ktr = [persist.tile([128, S], BF16, name=f"ktr{i}", tag=f"ktr{i}") for i in range(2)]
            aot = [persist.tile([128, S], BF16, name=f"aot{i}", tag=f"aot{i}") for i in range(2)]
            # V_ext: 16 seq tiles of (128, 4 heads * 128); per head the first
            # 64 cols are ALL ONES (they replicate the softmax denominator
            # onto PSUM partitions 0-63, where reciprocal_approx_fast is
            # legal) and cols 64-127 are V. The P @ V_ext matmul then yields
            # den on rows 0-63 and values on rows 64-127 of one PSUM tile —
            # no cross-partition broadcast needed (the old DRAM-bounce chain
            # cost ~2 DMAs + a copy per head-chunk on the critical tail).
            vext = [persist.tile([128, HL * 128], BF16, name=f"vext{i}", tag=f"vext{i}")
                    for i in range(S // 128)]
            for v in vext:
                nc.vector.memset(
                    v.rearrange("p (h c) -> p h c", c=128)[:, :, 0:64], 1.0)

            def emit_proj(sc, xt, warm_fill=False):
                """Q/K projections + RoPE and V projection for chunk sc.

                For chunk 0 the Q/K matmuls are emitted in two k-groups per
                (matrix, ft) so the PE can start on the first DMA'd halves of
                wq/wk/xt0 while the second halves stream in. (A warm_fill
                variant with gap-filler matmuls in the idle scores-PSUM slots
                between the two Q k-groups measured neutral: 185.1 vs the
                183.2-184.9 band.)"""
                s0 = sc * SC
                kgroups = ([range(0, KC // 2), range(KC // 2, KC)] if sc == 0
                           else [range(KC)])
                for w_sb, dst in ((wq_sb, qtr), (wk_sb, ktr)):
                    ps_ft = [mm_ps.tile([128, SC], F32, name="ps", tag="mm")
                             for _ in range(2)]
                    for kg in kgroups:
                        for ft in range(2):
                            ps = ps_ft[ft]
                            for k in kg:
                                nc.tensor.matmul(ps, w_sb[:, k, ft * 128:(ft + 1) * 128],
                                                 xt[:, k, :],
                                                 start=(k == 0), stop=(k == KC - 1))
                    for ft in range(2):
                        ps = ps_ft[ft]
                        qsw = rope_pool.tile([128, SC], F32, name="qsw", tag="qsw")
                        qcc = rope_pool.tile([128, SC], F32, name="qcc", tag="qcc")
                        # only the two PSUM readers (shuffle, cos-mul) stay on
                        # DVE — they gate the mm_ps tile release; the SBUF-only
                        # sin-mul and final add run on the idle GpSimd/Pool
                        # engine (it cannot touch PSUM), halving DVE's RoPE
                        # load so norm chains don't queue behind it
                        nc.vector.stream_shuffle(qsw, ps, SWAP16)
                        dslice = dst[ft][:, s0:s0 + SC]
                        nc.gpsimd.tensor_mul(qsw, qsw, ss_sb[:, s0:s0 + SC])
                        nc.vector.tensor_mul(qcc, ps, cc_sb[:, s0:s0 + SC])
                        nc.gpsimd.tensor_add(dslice, qcc, qsw)
                for st in range(4):
                    psv = mm_ps.tile([128, F], F32, name="psv", tag="mm")
                    for k in range(KC):
                        nc.tensor.matmul(psv, xt[:, k, st * 128:(st + 1) * 128],
                                         wv_sb[:, k, :],
                                         start=(k == 0), stop=(k == KC - 1))
                    v = vext[sc * 4 + st]
                    v3 = v.rearrange("p (h c) -> p h c", c=128)[:, :, 64:128]
                    p3 = psv.rearrange("p (h c) -> p h c", c=DH)
                    # DVE, not ACT: a Copy on the scalar engine would force
                    # an activation-table reload next to the Exp ops (~1.3 us
                    # each) and serialize the softmax pipeline behind it
                    nc.vector.tensor_copy(out=v3, in_=p3)

            def emit_prefetch(sc):
                xtn = xs_pool.tile([128, KC, SC], BF16, name="xtn", tag="xt")
                nc.gpsimd.dma_start(out=xtn, in_=xTr[:, :, sc * SC:(sc + 1) * SC])
                return xtn

            def emit_attn(sc):
                s0 = sc * SC
                nblocks = 4 * sc + 4
                for h in (1, 3, 0, 2):   # even heads last: their norm writes
                    ft, hr = h // 2, (h % 2) * DH      # skip the staging DMA
                    pv = pv_ps.tile([128, SC], F32, name="pv", tag="pv")
                    for pair in range(nblocks // 2):
                        sps = sc_ps.tile([128, 2 * SC], F32, name="sps", tag="sps")
                        w = []
                        for half in range(2):
                            j = 2 * pair + half
                            diag = j >= 4 * sc
                            t = j - 4 * sc
                            # queries left of the window are fully masked;
                            # clamp the trim at N=128 (PE-neutral vs N=256 —
                            # the ~107 ns LDWEIGHTS dominates either way —
                            # but trims 128 columns of exp per t=3 block)
                            w0 = min(128 * t, SC - 128) if diag else 0
                            w.append(w0)
                            tgt = sps[:, half * SC + w0:(half + 1) * SC]
                            nc.tensor.matmul(tgt,
                                             ktr[ft][hr:hr + DH, j * 128:(j + 1) * 128],
                                             qtr[ft][hr:hr + DH, s0 + w0:s0 + SC],
                                             start=True, stop=True)
                            if diag:
                                # -1e9 only where the staircase actually
                                # masks: columns [w0, 128(t+1)); columns
                                # right of the block's last key need no mask
                                me = 128 * (t + 1)
                                nc.tensor.matmul(
                                    sps[:, half * SC + w0:half * SC + me],
                                    eye_sb,
                                    gm_sb[:, SC - 128 * t + w0:SC + 128],
                                    start=False, stop=True,
                                    skip_group_check=True)
                        pt = p_pool.tile([128, 2 * SC], BF16, name="pt", tag="pt")
                        if w[0] == 0 and w[1] == 0:
                            nc.scalar.activation(out=pt, in_=sps,
                                                 func=mybir.ActivationFunctionType.Exp,
                                                 scale=0.125)
                        else:
                            for half in range(2):
                                sl = slice(half * SC + w[half], (half + 1) * SC)
                                nc.scalar.activation(out=pt[:, sl], in_=sps[:, sl],
                                                     func=mybir.ActivationFunctionType.Exp,
                                                     scale=0.125)
                        for half in range(2):
                            j = 2 * pair + half
                            w0 = w[half]
                            nc.tensor.matmul(pv[:, w0:SC],
                                             vext[j][:, h * 128:h * 128 + 128],
                                             pt[:, half * SC + w0:(half + 1) * SC],
                                             start=(j == 0), stop=(j == nblocks - 1))
                    # normalize: the ones-columns of V_ext put den replicated
                    # on pv rows 0-63 (values on 64-127), so the reciprocal
                    # runs directly at base partition 0 (PSUM input verified
                    # on HW) and the mul reads values at base partition 64 —
                    # no broadcast step. reciprocal_approx_fast keeps ~18
                    # bits; ACT Reciprocal would thrash the activation table
                    # next to the Exp ops.
                    rbc = stg_pool.tile([DH, SC], F32, name="rbc", tag="rbc")
                    nc.vector.reciprocal_approx_fast(out=rbc, in_=pv[0:DH, :])
                    if hr == 0:
                        nc.vector.tensor_mul(aot[ft][0:DH, s0:s0 + SC], pv[DH:128, :], rbc)
                    else:
                        stg = stg_pool.tile([DH, SC], BF16, name="stg", tag="stg")
                        nc.vector.tensor_mul(stg, pv[DH:128, :], rbc)
                        nc.gpsimd.dma_start(out=aot[ft][hr:hr + DH, s0:s0 + SC], in_=stg)

            def emit_wo(sc, act_copy=False):
                # the DRAM writes alternate gpsimd/sync queues so neither
                # queue's ~700 ns/issue engine time serializes the drain.
                # act_copy moves the PSUM->SBUF copies to the ACT engine
                # (Copy shares the Exp activation-table set): used for the
                # final chunk's Wo, where ACT is idle but the DVE still has
                # norm chains draining.
                s0 = sc * SC
                for st in range(4):
                    so = s0 + st * 128
                    for nn in range(2):
                        pw = mm_ps.tile([128, SC], F32, name="pw", tag="mm")
                        for ft in range(2):
                            nc.tensor.matmul(pw, aot[ft][:, so:so + 128],
                                             wo_sb[:, ft, nn * SC:(nn + 1) * SC],
                                             start=(ft == 0), stop=(ft == 1))
                        og = out_pool.tile([128, SC], BF16, name="og", tag="og")
                        if act_copy:
                            nc.scalar.activation(
                                out=og, in_=pw,
                                func=mybir.ActivationFunctionType.Copy)
                        else:
                            nc.vector.tensor_copy(out=og, in_=pw)
                        if nn == 0:
                            nc.gpsimd.dma_start(
                                out=out[so:so + 128, nn * SC:(nn + 1) * SC], in_=og)
                        else:
                            nc.sync.dma_start(
                                out=out[so:so + 128, nn * SC:(nn + 1) * SC], in_=og)

            # schedule: next chunk's projections run between attention(sc)
            # and Wo(sc) so the PE has work while the last head's
            # normalization chain drains. NOTE: a denser cross-rep pipelined
            # variant (proj(0) of rep r+1 hoisted before wo(3) of rep r)
            # reached 86.9% PE occupancy but measured consistently SLOWER —
            # the sustained draw downclocks the chip (P0) to ~2.0 GHz; the
            # small rep-boundary bubble keeps it at 2.4 GHz and wins.
            xt_first = xt0
            for rep in range(repeat):
                with nc.named_scope(f"r{rep}.proj0"):
                    emit_proj(0, xt_first, warm_fill=(rep == 0))
                    xt_next = emit_prefetch(1)
                for sc in range(NSC):
                    with nc.named_scope(f"r{rep}.attn{sc}"):
                        emit_attn(sc)
                    if sc == 0 and rep == 0:
                        emit_late_consts()
                    if sc + 1 < NSC:
                        xt_cur = xt_next
                        if sc + 2 < NSC:
                            xt_next = emit_prefetch(sc + 2)
                        with nc.named_scope(f"r{rep}.proj{sc + 1}"):
                            emit_proj(sc + 1, xt_cur)
                    # Wo filler placement: chunk 3's window is the PE's
                    # catch-all (attn3 + deferred Wo + next rep's proj0
                    # interleave there at ~100% PE); spreading Wo into the
                    # earlier windows was tried and measured SLOWER (178.5 vs
                    # 168.5 us) — the wo tiles rotate through the same mm_ps
                    # PSUM pool as the projections and push proj(sc+1) out.
                    if sc == 3:
                        with nc.named_scope(f"r{rep}.wo012"):
                            emit_wo(0)
                            emit_wo(1)
                            emit_wo(2)
                if rep == repeat - 1:
                    # keep-warm matmuls before the FINAL Wo: they fill the PE
                    # idle window while the last normalization chains drain
                    # so the clock gate stays at 2.4 GHz
                    for w in range(NTAIL):
                        wps = mm_ps.tile([128, SC], F32, name="wps", tag="mm")
                        nc.tensor.matmul(wps, zt[:, 0:128], zt, start=True, stop=True)
                with nc.named_scope(f"r{rep}.wo3"):
                    emit_wo(NSC - 1, act_copy=True)
                if rep + 1 < repeat:
                    xt_first = emit_prefetch(0)
            if debug:
                for i in range(2):
                    nc.sync.dma_start(out=qtrd[i], in_=qtr[i][:, :])
                    nc.sync.dma_start(out=ktrd[i], in_=ktr[i][:, :])
                    nc.sync.dma_start(out=aotd[i], in_=aot[i][:, :])
                for i in range(16):
                    nc.sync.dma_start(out=vexd[i], in_=vext[i][:, :])

    nc.compile()
    return nc


def _rope_tables():
    inv_freq = 1.0 / (THETA ** (np.arange(0, DH, 2, dtype=np.float64) / DH))  # (32,)
    ang = np.arange(S, dtype=np.float64)[:, None] * inv_freq[None, :]         # (S, 32)
    cos = np.cos(ang).T.astype(np.float32)                                    # (32, S)
    sin = np.sin(ang).T.astype(np.float32)
    # quadrant layout per head: [x1(f0:16); x2(f0:16); x1(f16:32); x2(f16:32)]
    cc64 = np.concatenate([cos[0:16], cos[0:16], cos[16:32], cos[16:32]], axis=0)
    ss64 = np.concatenate([-sin[0:16], sin[0:16], -sin[16:32], sin[16:32]], axis=0)
    # (64, S) bf16: rows repeat per 64 partitions; the device duplicates on
    # load. Q/K are bf16 anyway, so bf16 tables cost no extra accuracy but
    # halve the startup DMA bytes.
    return (np.ascontiguousarray(cc64).astype(NP_BF16),
            np.ascontiguousarray(ss64).astype(NP_BF16))


def _gmask():
    # gm[j, c] = NEG if j > c - SC else 0   (c in [0, 2*SC))
    j = np.arange(128)[:, None]
    c = np.arange(2 * SC)[None, :]
    return np.where(j > c - SC, np.float32(NEG), np.float32(0.0)).astype(NP_BF16)


def _perm_rows():
    # per head, per 32-row quadrant: 16 even pair-elements then their odds,
    # so the RoPE partner swap stays within a 32-partition stream_shuffle group
    p = []
    for h in range(HL):
        base = h * DH
        p.extend(base + np.arange(0, 32, 2))   # x1 of pairs 0..15
        p.extend(base + np.arange(1, 32, 2))   # x2 of pairs 0..15
        p.extend(base + 32 + np.arange(0, 32, 2))  # x1 of pairs 16..31
        p.extend(base + 32 + np.arange(1, 32, 2))  # x2 of pairs 16..31
    return np.array(p)


_NC_CACHE = {}


def make_in_maps(x, Wq, Wk, Wv, Wo):
    x = np.asarray(x, dtype=np.float32)
    Wq = np.asarray(Wq, dtype=np.float32)
    Wk = np.asarray(Wk, dtype=np.float32)
    Wv = np.asarray(Wv, dtype=np.float32)
    Wo = np.asarray(Wo, dtype=np.float32)

    cc, ss = _rope_tables()
    gm = _gmask()
    eye = np.eye(128, dtype=np.float32).astype(NP_BF16)
    perm = _perm_rows()

    in_maps = []
    for c in range(NCORES):
        b, g = c // NGROUPS, c % NGROUPS
        rows = slice(g * F, (g + 1) * F)
        wq_g = Wq[rows, :][perm, :]
        wk_g = Wk[rows, :][perm, :]
        in_maps.append({
            "xT": np.ascontiguousarray(x[b].T).astype(NP_BF16),
            "wqT": np.ascontiguousarray(wq_g.T).astype(NP_BF16),
            "wkT": np.ascontiguousarray(wk_g.T).astype(NP_BF16),
            "wvT": np.ascontiguousarray(Wv[rows, :].T).astype(NP_BF16),
            "woT": np.ascontiguousarray(Wo[:, rows].T).astype(NP_BF16),
            "cc": cc, "ss": ss, "gmask": gm, "eye": eye,
        })
    return in_maps


def combine_outputs(results, ncores=NCORES):
    out = np.zeros((B, S, D), dtype=np.float64)
    for c in range(ncores):
        out[c // NGROUPS] += results[c]["out"].astype(np.float64)
    return out.astype(np.float32)


def kernel(x, Wq, Wk, Wv, Wo):
    in_maps = make_in_maps(x, Wq, Wk, Wv, Wo)
    if "nc" not in _NC_CACHE:
        _NC_CACHE["nc"] = build_nc()
    nc = _NC_CACHE["nc"]
    res = run_bass_kernel_spmd(nc, in_maps, core_ids=list(range(NCORES)))
    return combine_outputs(res.results, NCORES)

